# revision 7
# baseline (speedup 1.0000x reference)
"""Trainium2 Bass kernel for nn_ContextEncoderModel (bi-LSTM context encoder).

Model: two independent BasicLSTMCells (left/right) over T=100 steps, take the
output at t=length-1 for each row, concat -> 3-layer relu MLP.

Sharding (8 cores = 4 pairs): pair p owns batch rows [64p, 64p+64).  Even core
of a pair runs the LEFT lstm for those rows, odd core the RIGHT lstm.  After
the recurrence each pair AllGathers its two last-outputs ([64,512] each) and
both cores compute the (tiny) MLP redundantly; the host takes the even core's
output.

Per-core device program (identical SPMD program, different data):
  - x-projection and h-recurrence matmuls accumulate in the same PSUM banks
    (float32r at 1 cycle/row).  x-part is batched 2 steps per PSUM window
    (M=128); h-part is per-step (M=64, N-bound so M-underuse is free).
  - W columns are host-permuted to [i f o j] blocks per 256-unit half so one
    sigmoid ACTIVATE covers i,f,o (forget bias folded into the bias row).
  - gates -> c,h on ACT/DVE; h is PE-transposed back to h^T for the next step.
  - h_t streamed to DRAM; last outputs gathered with an indirect DMA using
    host-precomputed row offsets; pair AllGather; MLP with PE transposes.
"""

import os
import sys

sys.path.insert(0, "/opt/trn_rl_repo")

import numpy as np

import concourse.bass as bass
import concourse.tile as tile
from concourse import bacc, mybir
from concourse.bass import IndirectOffsetOnAxis
from concourse.bass_utils import run_bass_kernel_spmd

F32 = mybir.dt.float32
F32R = mybir.dt.float32r
I32 = mybir.dt.int32

B, T, E, H, ENC = 256, 100, 300, 512, 1024
N_CORES = 8
BL = B // (N_CORES // 2)  # 64 batch rows per pair
EA = E + 1  # x rows augmented with ones row (bias)
G4 = 4 * H  # 2048 gate columns
KH = H // 128  # 4 h-contraction chunks
NT = G4 // 1024  # 2 half-window psum tiles


def _r(ap):
    return ap.bitcast(F32R)


def build_program(n_cores=N_CORES, t_steps=T):
    nc = bacc.Bacc("TRN2", target_bir_lowering=False, debug=False,
                   num_devices=n_cores)

    xT_d = nc.dram_tensor("xT", [EA, t_steps * BL], F32R, kind="ExternalInput").ap()
    wx_d = nc.dram_tensor("Wx", [EA, G4], F32R, kind="ExternalInput").ap()
    wh_d = nc.dram_tensor("Wh", [H, G4], F32R, kind="ExternalInput").ap()
    offs_d = nc.dram_tensor("offs", [BL, 1], I32, kind="ExternalInput").ap()
    ident_d = nc.dram_tensor("ident", [64, 64], F32, kind="ExternalInput").ap()
    tw_d = nc.dram_tensor("tw", [ENC, ENC], F32R, kind="ExternalInput").ap()
    hw0_d = nc.dram_tensor("hw0", [ENC, ENC], F32R, kind="ExternalInput").ap()
    hw1_d = nc.dram_tensor("hw1", [ENC, ENC], F32R, kind="ExternalInput").ap()
    out_d = nc.dram_tensor("out", [BL, ENC], F32, kind="ExternalOutput").ap()

    hstore_d = nc.dram_tensor("hstore", [t_steps * BL, H], F32).ap()

    groups = [[i, i + 1] for i in range(0, n_cores, 2)]

    with tile.TileContext(nc) as tc:
        with tc.tile_pool(name="misc", bufs=1) as misc:
            ident = misc.tile([64, 64], F32)
            nc.sync.dma_start(ident[:], ident_d[:])
            offs = misc.tile([BL, 1], I32)
            nc.sync.dma_start(offs[:], offs_d[:])
            c_st = misc.tile([BL, H], F32)
            nc.vector.memset(c_st[:], 0.0)

            with tc.tile_pool(name="lstm_w", bufs=1) as lw:
                # xT chunks: rows [0:128),[128:256) in one tile (col-stacked),
                # rows [256:301) in a second tile.
                xt01 = lw.tile([128, 2 * t_steps * BL], F32R)
                nc.sync.dma_start(xt01[:, 0:t_steps * BL], xT_d[0:128, :])
                nc.sync.dma_start(xt01[:, t_steps * BL:], xT_d[128:256, :])
                xt2 = lw.tile([EA - 256, t_steps * BL], F32R)
                nc.sync.dma_start(xt2[:], xT_d[256:EA, :])

                wx01 = lw.tile([128, 2 * G4], F32R)
                nc.sync.dma_start(wx01[:, 0:G4], wx_d[0:128, :])
                nc.sync.dma_start(wx01[:, G4:], wx_d[128:256, :])
                wx2 = lw.tile([EA - 256, G4], F32R)
                nc.sync.dma_start(wx2[:], wx_d[256:EA, :])

                wh = lw.tile([128, KH * G4], F32R)
                for k in range(KH):
                    nc.sync.dma_start(wh[:, k * G4:(k + 1) * G4],
                                      wh_d[k * 128:(k + 1) * 128, :])

                def x_lhsT(k, t):
                    """stationary [K,64] slice of x^T for step t, chunk k."""
                    c0 = t * BL
                    if k < 2:
                        return xt01[:, k * t_steps * BL + c0:
                                    k * t_steps * BL + c0 + BL]
                    return xt2[:, c0:c0 + BL]

                def x_rhs(k, c0, c1):
                    if k < 2:
                        return wx01[:, k * G4 + c0:k * G4 + c1]
                    return wx2[:, c0:c1]

                with (
                    tc.tile_pool(name="zps", bufs=3, space="PSUM") as zps,
                    tc.tile_pool(name="tps", bufs=2, space="PSUM") as tps,
                    tc.tile_pool(name="work", bufs=2) as wk,
                    tc.tile_pool(name="hT", bufs=3) as hTp,
                ):
                    hT_prev = hTp.tile([128, KH * 64], F32R, tag="hT")
                    nc.vector.memset(hT_prev[:].bitcast(F32), 0.0)

                    for t in range(t_steps):
                        # zA = [i(512) f(512)], zB = [o(512) j(512)]
                        zts = [zps.tile([64, 1024], F32, tag="z",
                                        name=f"z{t}_{i}")
                               for i in range(2)]
                        for ti in range(2):
                            for n in range(2):
                                c0 = ti * 1024 + n * 512
                                for k in range(3):
                                    nc.tensor.matmul(
                                        zts[ti][:, n * 512:(n + 1) * 512],
                                        x_lhsT(k, t),
                                        x_rhs(k, c0, c0 + 512),
                                        start=(k == 0), stop=False,
                                        skip_group_check=True)
                                for k in range(KH):
                                    nc.tensor.matmul(
                                        zts[ti][:, n * 512:(n + 1) * 512],
                                        hT_prev[:, k * 64:(k + 1) * 64],
                                        wh[:, k * G4 + c0:k * G4 + c0 + 512],
                                        start=False, stop=(k == KH - 1),
                                        skip_group_check=True)

                        sgA = wk.tile([BL, 1024], F32, tag="sgA")
                        nc.scalar.activation(
                            sgA[:], zts[0][:],
                            mybir.ActivationFunctionType.Sigmoid)
                        sgO = wk.tile([BL, 512], F32, tag="sgO")
                        nc.scalar.activation(
                            sgO[:], zts[1][:, 0:512],
                            mybir.ActivationFunctionType.Sigmoid)
                        tj = wk.tile([BL, 512], F32, tag="tj")
                        nc.scalar.activation(
                            tj[:], zts[1][:, 512:1024],
                            mybir.ActivationFunctionType.Tanh)
                        tmp = wk.tile([BL, 1024], F32, tag="tmp")
                        nc.vector.tensor_mul(tmp[:, 0:512], sgA[:, 0:512], tj[:])
                        nc.vector.tensor_mul(tmp[:, 512:1024], c_st[:],
                                             sgA[:, 512:1024])
                        nc.vector.tensor_add(c_st[:], tmp[:, 0:512],
                                             tmp[:, 512:1024])
                        th = wk.tile([BL, 512], F32, tag="th")
                        nc.scalar.activation(
                            th[:], c_st[:], mybir.ActivationFunctionType.Tanh)
                        h_t = wk.tile([BL, H], F32, tag="h")
                        nc.vector.tensor_mul(h_t[:], th[:], sgO[:])

                        # transpose h -> hT for next step (4 groups of 128)
                        tp = tps.tile([128, KH * 64], F32)
                        for g in range(KH):
                            nc.tensor.transpose(
                                tp[:, g * 64:(g + 1) * 64],
                                h_t[:, g * 128:(g + 1) * 128], ident[:])
                        hT_cur = hTp.tile([128, KH * 64], F32R, tag="hT")
                        nc.vector.tensor_copy(hT_cur[:], tp[:])
                        hT_prev = hT_cur

                        nc.sync.dma_start(
                            hstore_d[t * BL:(t + 1) * BL, :], h_t[:])

            # ---- last-output gather + pair AllGather + MLP ----
            with (
                tc.tile_pool(name="mlp", bufs=1) as mp,
                tc.tile_pool(name="mwork", bufs=2) as mw,
                tc.tile_pool(name="dram", bufs=1, space="DRAM") as dp,
                tc.tile_pool(name="yps", bufs=2, space="PSUM") as yps,
                tc.tile_pool(name="tps2", bufs=2, space="PSUM") as tps2,
            ):
                wts = []
                for wd in (tw_d, hw0_d, hw1_d):
                    wt = mp.tile([128, 8 * ENC], F32R)
                    for k in range(8):
                        nc.sync.dma_start(wt[:, k * ENC:(k + 1) * ENC],
                                          wd[k * 128:(k + 1) * 128, :])
                    wts.append(wt)

                last = mp.tile([BL, H], F32)
                nc.gpsimd.indirect_dma_start(
                    out=last[:], out_offset=None, in_=hstore_d[:],
                    in_offset=IndirectOffsetOnAxis(ap=offs[:, 0:1], axis=0))

                cc_in = dp.tile([BL, H], F32)
                nc.sync.dma_start(cc_in[:], last[:])
                cc_out = dp.tile([2, BL, H], F32)
                nc.gpsimd.collective_compute(
                    "AllGather", mybir.AluOpType.bypass,
                    replica_groups=groups,
                    ins=[cc_in[:].opt()], outs=[cc_out[:].opt()])

                x_sb = mp.tile([BL, ENC], F32)
                nc.sync.dma_start(x_sb[:, 0:H], cc_out[0])
                nc.sync.dma_start(x_sb[:, H:ENC], cc_out[1])

                for li in range(3):
                    # transpose x [64,1024] -> xT chunks [128,64] x 8
                    xT_sb = mw.tile([128, 512], F32R, tag="xT")
                    for hf in range(2):
                        tp = tps2.tile([128, 256], F32)
                        for g in range(4):
                            k = hf * 4 + g
                            nc.tensor.transpose(
                                tp[:, g * 64:(g + 1) * 64],
                                x_sb[:, k * 128:(k + 1) * 128], ident[:])
                        nc.vector.tensor_copy(
                            xT_sb[:, hf * 256:(hf + 1) * 256], tp[:])
                    y = yps.tile([64, ENC], F32)
                    for n in range(2):
                        for k in range(8):
                            nc.tensor.matmul(
                                y[:, n * 512:(n + 1) * 512],
                                xT_sb[:, k * 64:(k + 1) * 64],
                                wts[li][:, k * ENC + n * 512:
                                        k * ENC + n * 512 + 512],
                                start=(k == 0), stop=(k == 7),
                                skip_group_check=True)
                    x_sb = mw.tile([BL, ENC], F32, tag="xact")
                    nc.scalar.activation(x_sb[:], y[:],
                                         mybir.ActivationFunctionType.Relu)

                nc.sync.dma_start(out_d[:], x_sb[:])

    nc.compile()
    return nc


def _permute_cols(w):
    """[rows, 2048] gate cols (i,j,f,o)*512 -> [i f o j]*512."""
    i, j, f, o = np.split(w, 4, axis=1)
    return np.ascontiguousarray(np.concatenate([i, f, o, j], axis=1))


def _prep_core(embed_rows, lengths_rows, w, b, tw, hw, t_steps=T):
    """Build the per-core input map (host-side layout transforms)."""
    x = np.ascontiguousarray(
        embed_rows.transpose(2, 1, 0).reshape(E, t_steps * BL))
    xT = np.concatenate([x, np.ones((1, t_steps * BL), np.float32)], axis=0)

    b = b.astype(np.float32).copy()
    b[2 * H:3 * H] += 1.0  # fold forget bias into the f-gate bias
    wx = np.concatenate([w[:E, :], b[None, :]], axis=0)

    return {
        "xT": np.ascontiguousarray(xT, np.float32),
        "Wx": _permute_cols(wx.astype(np.float32)),
        "Wh": _permute_cols(w[E:, :].astype(np.float32)),
        "offs": ((lengths_rows.astype(np.int64) - 1) * BL
                 + np.arange(BL)).astype(np.int32)[:, None],
        "ident": np.eye(64, dtype=np.float32),
        "tw": np.ascontiguousarray(tw, np.float32),
        "hw0": np.ascontiguousarray(hw[0], np.float32),
        "hw1": np.ascontiguousarray(hw[1], np.float32),
    }


_NC_CACHE = {}


def _get_program():
    if "nc" not in _NC_CACHE:
        _NC_CACHE["nc"] = build_program()
    return _NC_CACHE["nc"]


def make_in_maps(left_embed, left_lengths, right_embed, right_lengths,
                 W_left, b_left, W_right, b_right, trans_weights,
                 hidden_weights):
    in_maps = []
    for p in range(N_CORES // 2):
        rows = slice(p * BL, (p + 1) * BL)
        in_maps.append(_prep_core(left_embed[rows], left_lengths[rows],
                                  W_left, b_left, trans_weights,
                                  hidden_weights))
        in_maps.append(_prep_core(right_embed[rows], right_lengths[rows],
                                  W_right, b_right, trans_weights,
                                  hidden_weights))
    return in_maps


def kernel(left_embed, left_lengths, right_embed, right_lengths,
           W_left, b_left, W_right, b_right, trans_weights, hidden_weights):
    nc = _get_program()
    in_maps = make_in_maps(
        np.asarray(left_embed), np.asarray(left_lengths),
        np.asarray(right_embed), np.asarray(right_lengths),
        np.asarray(W_left), np.asarray(b_left),
        np.asarray(W_right), np.asarray(b_right),
        np.asarray(trans_weights), np.asarray(hidden_weights))
    res = run_bass_kernel_spmd(nc, in_maps, core_ids=list(range(N_CORES)))
    out = np.empty((B, ENC), np.float32)
    for p in range(N_CORES // 2):
        out[p * BL:(p + 1) * BL] = res.results[2 * p]["out"]
    return out
